# revision 8
# baseline (speedup 1.0000x reference)
"""Trainium2 Bass kernel for nn_BidirectionalAttention (LayerNorm -> QKV -> RoPE ->
attention with 16 persistent-memory KV tokens -> out projection).

Sharding: 8 cores = (batch b=2) x (4 head-pairs). Each core computes its batch's
LayerNorm + QKV for its 2 heads, full attention over n=4096 (+16 pm) keys, and a
partial output projection; the host sums the 4 partials per batch.

Self-contained: hardcodes all shapes from the problem spec.
"""
import sys

sys.path.insert(0, "/opt/trn_rl_repo")

import numpy as np
import ml_dtypes

import concourse.bass as bass
import concourse.tile as tile
from concourse import mybir
from concourse.bass_utils import run_bass_kernel_spmd

BF16 = mybir.dt.bfloat16
F32 = mybir.dt.float32
AF = mybir.ActivationFunctionType
OP = mybir.AluOpType

B, N, D = 2, 4096, 512
H, DH, NPM = 8, 64, 16
BASE, EPS = 10000.0, 1e-5
SCALE = DH ** -0.5
NCORES = 8
M_TOT = N + NPM          # 4112 keys
MCHUNKS = 33             # 32 seq chunks of 128 + 1 pm chunk of 16
NGRP = 11                # exp/AV groups of 3 m-chunks
QC = 8                   # query chunks of 512
NBLK = 32                # n blocks of 128


def _split_excess_waits(nc, max_waits=1):
    """walrus in this container rejects >1 sync waits per instruction; hoist
    extras onto same-engine nops inserted just before (same sequencer order)."""
    cnt = 0
    for fn in nc.m.functions:
        for bb in fn.blocks:
            insts = bb.instructions
            i = 0
            while i < len(insts):
                inst = insts[i]
                si = inst.sync_info
                if si is not None and si.on_wait is not None and len(si.on_wait) > max_waits:
                    waits = list(si.on_wait)
                    extra, keep = waits[:-max_waits], waits[-max_waits:]
                    nops = []
                    for j in range(0, len(extra), max_waits):
                        cnt += 1
                        nop = mybir.InstNoOp(name=f"I-waitsplit-{cnt}-{inst.name}",
                                             engine=inst.engine, ins=[], outs=[])
                        nop.sync_info = mybir.SyncInfo(on_wait=extra[j:j + max_waits],
                                                       on_update=[])
                        nc.register_instruction(nop, overwrite=True)
                        nops.append(nop)
                    si.on_wait = keep
                    for k, nop in enumerate(nops):
                        insts.insert(i + k, nop)
                    i += len(nops)
                i += 1
    return cnt


def build(reps=1):
    nc = bass.Bass()

    x_in = nc.dram_tensor("x_in", [N, D], BF16, kind="ExternalInput")
    wq_in = nc.dram_tensor("wq_in", [5 * 128, 384], BF16, kind="ExternalInput")
    wo0_in = nc.dram_tensor("wo0_in", [64, 512], BF16, kind="ExternalInput")
    wo1_in = nc.dram_tensor("wo1_in", [64, 512], BF16, kind="ExternalInput")
    cos_in = nc.dram_tensor("cos_in", [128, N], BF16, kind="ExternalInput")
    sin_in = nc.dram_tensor("sin_in", [128, N], BF16, kind="ExternalInput")
    pmk_in = nc.dram_tensor("pmk_in", [128, NPM], BF16, kind="ExternalInput")
    pmv_in = nc.dram_tensor("pmv_in", [NPM, 130], BF16, kind="ExternalInput")
    msk_in = nc.dram_tensor("msk_in", [128, NBLK], F32, kind="ExternalInput")
    out_p = nc.dram_tensor("out_p", [N, D], F32, kind="ExternalOutput")

    with tile.TileContext(nc) as tc:
      for _rep in range(reps):
        with (
            tc.tile_pool(name="persist", bufs=1) as pers,
            tc.tile_pool(name="okpool", bufs=16) as okpool,
            tc.tile_pool(name="dram", bufs=1, space="DRAM") as dr,
        ):
            w_sb = pers.tile([128, 5, 384], BF16)
            nc.sync.dma_start(out=w_sb, in_=wq_in.rearrange("(kc p) m -> p kc m", p=128))
            wo0_sb = pers.tile([64, 512], BF16)
            wo1_sb = pers.tile([64, 512], BF16)
            nc.sync.dma_start(out=wo0_sb, in_=wo0_in[:, :])
            nc.sync.dma_start(out=wo1_sb, in_=wo1_in[:, :])
            msk_sb = pers.tile([128, NBLK], F32)
            nc.sync.dma_start(out=msk_sb, in_=msk_in[:, :])
            eps_sb = pers.tile([128, 1], F32)
            nc.vector.memset(eps_sb, EPS)
            QT = pers.tile([128, N], BF16)       # [q_g0(64); q_g1(64)] x n
            KT = pers.tile([128, M_TOT], BF16)   # [k_g0(64); k_g1(64)] x (n + pm)
            Vnat = pers.tile([128, MCHUNKS, 130], BF16)  # per m-chunk: [v_g0(64), m, v_g1(64), m]
            nc.sync.dma_start(out=KT[:, N:M_TOT], in_=pmk_in[:, :])
            nc.sync.dma_start(out=Vnat[0:NPM, 32, :], in_=pmv_in[:, :])
            xn_dram = dr.tile([N, D], BF16)

            # ---------------- P0-P2: load x, LayerNorm (in place), DRAM roundtrip transpose
            with tc.tile_pool(name="xpool", bufs=1) as xp, tc.tile_pool(name="lnp", bufs=4) as lnp:
                xt = xp.tile([128, NBLK, D], BF16)
                nc.sync.dma_start(out=xt, in_=x_in.rearrange("(t p) d -> p t d", p=128))
                for t in range(NBLK):
                    stats = lnp.tile([128, 6], F32, tag="stats")
                    nc.vector.bn_stats(out=stats, in_=xt[:, t, :])
                    mv = lnp.tile([128, 2], F32, tag="mv")
                    nc.vector.bn_aggr(out=mv, in_=stats)
                    rstd = lnp.tile([128, 1], F32, tag="rstd")
                    nc.scalar.activation(out=rstd, in_=mv[:, 1:2], func=AF.Sqrt,
                                         bias=eps_sb, scale=1.0)
                    nc.vector.reciprocal(out=rstd, in_=rstd)
                    nc.vector.tensor_scalar(out=xt[:, t, :], in0=xt[:, t, :],
                                            scalar1=mv[:, 0:1], scalar2=rstd,
                                            op0=OP.subtract, op1=OP.mult)
                nc.sync.dma_start(out=xn_dram.rearrange("(t p) d -> p t d", p=128), in_=xt)

            with tc.tile_pool(name="xnt", bufs=1) as xntp:
                xnT = []
                for kc in range(4):
                    t_ = xntp.tile([128, N], BF16, tag=f"xnT{kc}")
                    nc.sync.dma_start_transpose(t_, xn_dram[:, kc * 128:(kc + 1) * 128])
                    xnT.append(t_)
                ones_row = xntp.tile([128, N], BF16, tag="ones")
                nc.vector.memset(ones_row, 0.0)
                nc.vector.memset(ones_row[0:1, :], 1.0)
                xnT.append(ones_row)

                # ---------------- P3-P4: QKV^T GEMM for q/k cols + RoPE + assembly
                with tc.tile_pool(name="rope", bufs=1) as rp, \
                     tc.tile_pool(name="qkps", bufs=3, space="PSUM") as qkps:
                    cos_sb = rp.tile([128, N], BF16, tag="cos")
                    sin_sb = rp.tile([128, N], BF16, tag="sin")
                    nc.sync.dma_start(out=cos_sb, in_=cos_in[:, :])
                    nc.sync.dma_start(out=sin_sb, in_=sin_in[:, :])
                    A = rp.tile([128, N], BF16, tag="A")
                    Bt = rp.tile([128, N], BF16, tag="B")
                    for mi, dst in ((0, A), (1, Bt)):
                        for nc8 in range(QC):
                            psq = qkps.tile([128, 512], F32, tag="qk")
                            for kc in range(5):
                                nc.tensor.matmul(psq, w_sb[:, kc, mi * 128:(mi + 1) * 128],
                                                 xnT[kc][:, nc8 * 512:(nc8 + 1) * 512],
                                                 start=(kc == 0), stop=(kc == 4))
                            eng = nc.vector if (nc8 % 2 == 0) else nc.scalar
                            if eng is nc.vector:
                                nc.vector.tensor_copy(out=dst[:, nc8 * 512:(nc8 + 1) * 512], in_=psq)
                            else:
                                nc.scalar.copy(out=dst[:, nc8 * 512:(nc8 + 1) * 512], in_=psq)
                    # RoPE: rotA = A*cos - B*sin ; rotB = B*cos + A*sin  (overwrite A/B)
                    t1 = rp.tile([128, N], BF16, tag="t1")
                    t2 = rp.tile([128, N], BF16, tag="t2")
                    t3 = rp.tile([128, N], BF16, tag="t3")
                    t4 = rp.tile([128, N], BF16, tag="t4")
                    nc.vector.tensor_tensor(out=t1, in0=A, in1=cos_sb, op=OP.mult)
                    nc.vector.tensor_tensor(out=t2, in0=Bt, in1=sin_sb, op=OP.mult)
                    nc.vector.tensor_tensor(out=t3, in0=Bt, in1=cos_sb, op=OP.mult)
                    nc.vector.tensor_tensor(out=t4, in0=A, in1=sin_sb, op=OP.mult)
                    nc.vector.tensor_tensor(out=A, in0=t1, in1=t2, op=OP.subtract)
                    nc.vector.tensor_tensor(out=Bt, in0=t3, in1=t4, op=OP.add)
                    # assemble QT/KT (dim order per head: [first32, second32])
                    nc.vector.tensor_copy(out=QT[0:32, :], in_=A[0:32, :])
                    nc.vector.tensor_copy(out=QT[32:64, :], in_=Bt[0:32, :])
                    nc.vector.tensor_copy(out=QT[64:96, :], in_=A[32:64, :])
                    nc.vector.tensor_copy(out=QT[96:128, :], in_=Bt[32:64, :])
                    nc.vector.tensor_copy(out=KT[0:32, 0:N], in_=A[64:96, :])
                    nc.vector.tensor_copy(out=KT[32:64, 0:N], in_=Bt[64:96, :])
                    nc.vector.tensor_copy(out=KT[64:96, 0:N], in_=A[96:128, :])
                    nc.vector.tensor_copy(out=KT[96:128, 0:N], in_=Bt[96:128, :])

                # ---------------- P5: V natural GEMM (+ mask fold + ones cols)
                msk3 = msk_sb[:, :].rearrange("p (c one) -> p c one", one=1)
                nc.vector.tensor_copy(out=Vnat[:, 0:NBLK, 64:65], in_=msk3)
                nc.vector.tensor_copy(out=Vnat[:, 0:NBLK, 129:130], in_=msk3)
                with tc.tile_pool(name="vps", bufs=3, space="PSUM") as vps:
                    for nb in range(NBLK):
                        psv = vps.tile([128, 128], F32, tag="v")
                        for kc in range(5):
                            nc.tensor.matmul(psv, xnT[kc][:, nb * 128:(nb + 1) * 128],
                                             w_sb[:, kc, 256:384],
                                             start=(kc == 0), stop=(kc == 4))
                        dst = Vnat[:, nb, :].rearrange("p (g c) -> p g c", c=65)[:, :, 0:64]
                        src = psv.rearrange("p (g c) -> p g c", c=64)
                        nc.vector.tensor_scalar(out=dst, in0=src,
                                                scalar1=msk_sb[:, nb:nb + 1], scalar2=None,
                                                op0=OP.mult, op1=OP.bypass)

            # ---------------- P6: attention
            opks = []
            with (
                tc.tile_pool(name="spool", bufs=2, space="PSUM") as spool,
                tc.tile_pool(name="opool", bufs=2, space="PSUM") as opool,
                tc.tile_pool(name="ppool", bufs=3) as ppool,
                tc.tile_pool(name="rpool", bufs=4) as rpool,
                tc.tile_pool(name="rbpool", bufs=2) as rbpool,
                tc.tile_pool(name="rdram", bufs=4, space="DRAM") as rdram,
            ):
                for qc in range(QC):
                    qsl = slice(qc * 512, (qc + 1) * 512)
                    opk_pair = []
                    for h in range(2):
                        hsl = slice(h * 64, (h + 1) * 64)
                        qh = QT[hsl, qsl]
                        o_ps = opool.tile([65, 512], F32, tag="o")
                        for g in range(NGRP):
                            sgrp = spool.tile([128, 1536], F32, tag="s")
                            for j in range(3):
                                mc = 3 * g + j
                                js = slice(j * 512, (j + 1) * 512)
                                if mc < 32:
                                    nc.tensor.matmul(sgrp[:, js],
                                                     KT[hsl, mc * 128:(mc + 1) * 128],
                                                     qh, start=True, stop=True)
                                else:
                                    nc.tensor.matmul(sgrp[0:NPM, js],
                                                     KT[hsl, N:M_TOT],
                                                     qh, start=True, stop=True)
                            pgrp = ppool.tile([128, 1536], BF16, tag="p")
                            if g < NGRP - 1:
                                nc.scalar.activation(out=pgrp, in_=sgrp, func=AF.Exp, scale=SCALE)
                            else:
                                nc.scalar.activation(out=pgrp[:, 0:1024], in_=sgrp[:, 0:1024],
                                                     func=AF.Exp, scale=SCALE)
                                nc.scalar.activation(out=pgrp[0:NPM, 1024:1536],
                                                     in_=sgrp[0:NPM, 1024:1536],
                                                     func=AF.Exp, scale=SCALE)
                            for j in range(3):
                                mc = 3 * g + j
                                js = slice(j * 512, (j + 1) * 512)
                                if mc < 32:
                                    nc.tensor.matmul(o_ps, Vnat[:, mc, 65 * h:65 * h + 65],
                                                     pgrp[:, js],
                                                     start=(mc == 0), stop=(mc == 32))
                                else:
                                    nc.tensor.matmul(o_ps, Vnat[0:NPM, mc, 65 * h:65 * h + 65],
                                                     pgrp[0:NPM, js],
                                                     start=False, stop=True)
                        # normalize: r = 1/denom ; broadcast via DRAM bounce ; opk_h = numer * r
                        r_sb = rpool.tile([1, 512], F32, tag="r")
                        nc.vector.reciprocal(out=r_sb, in_=o_ps[64:65, :])
                        r_dr = rdram.tile([1, 512], F32, tag="rd")
                        nc.sync.dma_start(out=r_dr[:, :], in_=r_sb)
                        rd_ap = r_dr[:, :]
                        r_bc = bass.AP(tensor=rd_ap.tensor, offset=rd_ap.offset,
                                       ap=[[0, 64]] + list(rd_ap.ap[1:]))
                        r64 = rbpool.tile([64, 512], F32, tag="rb")
                        nc.gpsimd.dma_start(out=r64, in_=r_bc)
                        opk_h = okpool.tile([64, 512], BF16, tag="ok")
                        nc.vector.tensor_tensor(out=opk_h, in0=o_ps[0:64, :], in1=r64,
                                                op=OP.mult)
                        opk_pair.append(opk_h)
                    opks.append(opk_pair)

            # ---------------- P7: output projection (K split per head)
            with (
                tc.tile_pool(name="fps", bufs=2, space="PSUM") as fps,
                tc.tile_pool(name="fsb", bufs=3) as fsb,
            ):
                for qc in range(QC):
                    for mb in range(4):
                        psf = fps.tile([128, 512], F32, tag="f")
                        nc.tensor.matmul(psf, opks[qc][0][:, mb * 128:(mb + 1) * 128],
                                         wo0_sb, start=True, stop=False)
                        nc.tensor.matmul(psf, opks[qc][1][:, mb * 128:(mb + 1) * 128],
                                         wo1_sb, start=False, stop=True)
                        f_sb = fsb.tile([128, 512], F32, tag="fs")
                        if (qc * 4 + mb) % 2 == 0:
                            nc.scalar.copy(out=f_sb, in_=psf)
                        else:
                            nc.vector.tensor_copy(out=f_sb, in_=psf)
                        row0 = qc * 512 + mb * 128
                        nc.sync.dma_start(out=out_p[row0:row0 + 128, :], in_=f_sb)

    _split_excess_waits(nc)
    return nc


_STATE = {}


def _get_nc():
    if "nc" not in _STATE:
        _STATE["nc"] = build()
    return _STATE["nc"]


def _rope_tables():
    inv = 1.0 / (BASE ** (np.arange(0, DH, 2, dtype=np.float64) / DH))  # [32]
    pos = np.arange(N, dtype=np.float64)
    fr = pos[None, :] * inv[:, None]                                   # [32, N]
    cos32 = np.cos(fr)
    sin32 = np.sin(fr)
    cosf = np.tile(cos32, (4, 1)).astype(ml_dtypes.bfloat16)
    sinf = np.tile(sin32, (4, 1)).astype(ml_dtypes.bfloat16)
    return cosf, sinf


def kernel(x, mask, ln_w, ln_b, w_qkv, w_out, pm):
    bf = ml_dtypes.bfloat16
    f = np.float32
    x = np.asarray(x, f)
    mask_b = np.asarray(mask).astype(bool)
    ln_w = np.asarray(ln_w, f)
    ln_b = np.asarray(ln_b, f)
    w_qkv = np.asarray(w_qkv, f)
    w_out = np.asarray(w_out, f)
    pm = np.asarray(pm, f)

    cosf, sinf = _rope_tables()
    w_eff = ln_w[:, None] * w_qkv                     # [512, 1536]
    brow = ln_b @ w_qkv                               # [1536]

    in_maps = []
    for c in range(NCORES):
        bc = c // 4
        g0, g1 = (c % 4) * 2, (c % 4) * 2 + 1
        qcols = lambda g, lo, hi: np.arange(g * 64 + lo, g * 64 + hi)
        acols = np.concatenate([qcols(g0, 0, 32), qcols(g1, 0, 32),
                                512 + qcols(g0, 0, 32), 512 + qcols(g1, 0, 32)])
        bcols = np.concatenate([qcols(g0, 32, 64), qcols(g1, 32, 64),
                                512 + qcols(g0, 32, 64), 512 + qcols(g1, 32, 64)])
        vcols = np.concatenate([1024 + qcols(g0, 0, 64), 1024 + qcols(g1, 0, 64)])
        cols = np.concatenate([acols, bcols, vcols])  # [384]
        wq = np.zeros((5 * 128, 384), f)
        wq[0:512] = w_eff[:, cols]
        wq[512] = brow[cols]
        # pm[0, g] : [NPM, 64] -> K^T rows = dims, cols = pm idx
        pmk = np.concatenate([pm[0, g0].T, pm[0, g1].T], axis=0)  # [128, NPM]
        pmv = np.zeros((NPM, 130), f)
        pmv[:, 0:64] = pm[1, g0]
        pmv[:, 64] = 1.0
        pmv[:, 65:129] = pm[1, g1]
        pmv[:, 129] = 1.0
        mk = mask_b[bc, 0].astype(f).reshape(NBLK, 128).T  # [128, NBLK]
        in_maps.append(dict(
            x_in=x[bc].astype(bf),
            wq_in=wq.astype(bf),
            wo0_in=w_out[g0 * 64:(g0 + 1) * 64].astype(bf),
            wo1_in=w_out[g1 * 64:(g1 + 1) * 64].astype(bf),
            cos_in=cosf, sin_in=sinf,
            pmk_in=pmk.astype(bf), pmv_in=pmv.astype(bf),
            msk_in=np.ascontiguousarray(mk),
        ))

    global _LAST_IN_MAPS
    _LAST_IN_MAPS = in_maps
    nc = _get_nc()
    res = run_bass_kernel_spmd(nc, in_maps, core_ids=list(range(NCORES)))
    out = np.zeros((B, N, D), f)
    for c in range(NCORES):
        out[c // 4] += res.results[c]["out_p"]
    return out


# revision 29
# speedup vs baseline: 335.0514x; 335.0514x over previous
"""Trainium2 Bass kernel for nn_BidirectionalAttention (LayerNorm -> QKV -> RoPE ->
attention with 16 persistent-memory KV tokens -> out projection).

Sharding: 8 cores = (batch b=2) x (4 head-pairs). Each core computes its batch's
LayerNorm + QKV for its 2 heads, full attention over n=4096 (+16 pm) keys, and a
partial output projection; the host sums the 4 partials per batch.

Self-contained: hardcodes all shapes from the problem spec.
"""
import sys

sys.path.insert(0, "/opt/trn_rl_repo")

import numpy as np
import ml_dtypes

import concourse.bass as bass
import concourse.tile as tile
from concourse import mybir
from concourse.bass_utils import run_bass_kernel_spmd

BF16 = mybir.dt.bfloat16
F32 = mybir.dt.float32
AF = mybir.ActivationFunctionType
OP = mybir.AluOpType

B, N, D = 2, 4096, 512
H, DH, NPM = 8, 64, 16
BASE, EPS = 10000.0, 1e-5
SCALE = DH ** -0.5
NCORES = 8
M_TOT = N + NPM          # 4112 keys
MCHUNKS = 33             # 32 seq chunks of 128 + 1 pm chunk of 16
NGRP = 11                # exp/AV groups of 3 m-chunks
QC = 8                   # query chunks of 512
NBLK = 32                # n blocks of 128


def _split_excess_waits(nc, max_waits=1):
    """walrus in this container rejects >1 sync waits per instruction; hoist
    extras onto same-engine nops inserted just before (same sequencer order)."""
    cnt = 0
    for fn in nc.m.functions:
        for bb in fn.blocks:
            insts = bb.instructions
            i = 0
            while i < len(insts):
                inst = insts[i]
                si = inst.sync_info
                if si is not None and si.on_wait is not None and len(si.on_wait) > max_waits:
                    waits = list(si.on_wait)
                    extra, keep = waits[:-max_waits], waits[-max_waits:]
                    nops = []
                    for j in range(0, len(extra), max_waits):
                        cnt += 1
                        nop = mybir.InstNoOp(name=f"I-waitsplit-{cnt}-{inst.name}",
                                             engine=inst.engine, ins=[], outs=[])
                        nop.sync_info = mybir.SyncInfo(on_wait=extra[j:j + max_waits],
                                                       on_update=[])
                        nc.register_instruction(nop, overwrite=True)
                        nops.append(nop)
                    si.on_wait = keep
                    for k, nop in enumerate(nops):
                        insts.insert(i + k, nop)
                    i += len(nops)
                i += 1
    return cnt


def build(reps=1):
    nc = bass.Bass()

    x_in = nc.dram_tensor("x_in", [N, D], BF16, kind="ExternalInput")
    wq_in = nc.dram_tensor("wq_in", [5 * 128, 384], BF16, kind="ExternalInput")
    wo0_in = nc.dram_tensor("wo0_in", [64, 512], BF16, kind="ExternalInput")
    wo1_in = nc.dram_tensor("wo1_in", [64, 512], BF16, kind="ExternalInput")
    cos_in = nc.dram_tensor("cos_in", [128, N], BF16, kind="ExternalInput")
    sin_in = nc.dram_tensor("sin_in", [128, N], BF16, kind="ExternalInput")
    pmk_in = nc.dram_tensor("pmk_in", [128, NPM], BF16, kind="ExternalInput")
    pmv_in = nc.dram_tensor("pmv_in", [NPM, 130], BF16, kind="ExternalInput")
    msk_in = nc.dram_tensor("msk_in", [128, NBLK], F32, kind="ExternalInput")
    ones_in = nc.dram_tensor("ones_in", [1, N], BF16, kind="ExternalInput")
    out_p = nc.dram_tensor("out_p", [N, D], F32, kind="ExternalOutput")

    import contextlib
    with tile.TileContext(nc) as tc:
      with (tc.For_i(0, reps, 1) if reps > 1 else contextlib.nullcontext()):
        with (
            tc.tile_pool(name="persist", bufs=1) as pers,
            tc.tile_pool(name="okpool", bufs=16) as okpool,
            tc.tile_pool(name="dram", bufs=1, space="DRAM") as dr,
        ):
            w_sb = pers.tile([128, 5, 384], BF16)
            nc.sync.dma_start(out=w_sb, in_=wq_in.rearrange("(kc p) m -> p kc m", p=128))
            wo0_sb = pers.tile([64, 512], BF16)
            wo1_sb = pers.tile([64, 512], BF16)
            nc.sync.dma_start(out=wo0_sb, in_=wo0_in[:, :])
            nc.sync.dma_start(out=wo1_sb, in_=wo1_in[:, :])
            msk_sb = pers.tile([128, NBLK], F32)
            nc.sync.dma_start(out=msk_sb, in_=msk_in[:, :])
            eps_sb = pers.tile([128, 1], F32)
            nc.vector.memset(eps_sb, EPS)
            QT = pers.tile([128, N], BF16)       # [q_g0(64); q_g1(64)] x n
            KT = pers.tile([128, M_TOT], BF16)   # [k_g0(64); k_g1(64)] x (n + pm)
            Vnat = pers.tile([128, MCHUNKS, 130], BF16)  # per m-chunk: [v_g0(64), m, v_g1(64), m]
            nc.sync.dma_start(out=KT[:, N:M_TOT], in_=pmk_in[:, :])
            nc.sync.dma_start(out=Vnat[0:NPM, 32, :], in_=pmv_in[:, :])
            xn_dram = dr.tile([N, D], BF16)

            # ---------------- P0-P2: load x, LayerNorm (in place), DRAM roundtrip transpose
            # chunked by groups of 8 n-tiles so DMA / LN / store / transpose stream
            with tc.tile_pool(name="xpool", bufs=1) as xp, tc.tile_pool(name="lnp", bufs=4) as lnp:
                xr = x_in.rearrange("(t p) d -> p t d", p=128)
                xnr = xn_dram.rearrange("(t p) d -> p t d", p=128)
                xch = []
                for ch in range(4):
                    xt = xp.tile([128, 8, D], BF16, tag=f"x{ch}")
                    nc.sync.dma_start(out=xt, in_=xr[:, ch * 8:(ch + 1) * 8, :])
                    xch.append(xt)
                scr = xp.tile([128, D], BF16, tag="scr")  # ACT-stats discard buffer
                inv_d = 1.0 / D
                for ch in range(4):
                    xt = xch[ch]
                    for tt in range(8):
                        mv = lnp.tile([128, 2], F32, tag="mv")
                        rstd = lnp.tile([128, 1], F32, tag="rstd")
                        if tt % 2 == 0:  # split LN stats between DVE and ACT
                            # DVE stats path
                            stats = lnp.tile([128, 6], F32, tag="stats")
                            nc.vector.bn_stats(out=stats, in_=xt[:, tt, :])
                            nc.vector.bn_aggr(out=mv, in_=stats)
                            nc.scalar.activation(out=rstd, in_=mv[:, 1:2], func=AF.Sqrt,
                                                 bias=eps_sb, scale=1.0)
                        else:
                            # ACT stats path: sum via Copy+accum, sumsq via Square+accum
                            sums = lnp.tile([128, 2], F32, tag="sums")
                            nc.scalar.activation(out=scr, in_=xt[:, tt, :], func=AF.Copy,
                                                 accum_out=sums[:, 0:1])
                            nc.scalar.activation(out=scr, in_=xt[:, tt, :], func=AF.Square,
                                                 accum_out=sums[:, 1:2])
                            nc.vector.tensor_scalar(out=mv, in0=sums, scalar1=inv_d,
                                                    scalar2=None, op0=OP.mult, op1=OP.bypass)
                            mu2 = lnp.tile([128, 1], F32, tag="mu2")
                            nc.vector.tensor_tensor(out=mu2, in0=mv[:, 0:1], in1=mv[:, 0:1],
                                                    op=OP.mult)
                            # var = meansq - mu^2 ; rstd = 1/sqrt(var+eps)
                            nc.vector.tensor_tensor(out=mv[:, 1:2], in0=mv[:, 1:2], in1=mu2,
                                                    op=OP.subtract)
                            nc.scalar.activation(out=rstd, in_=mv[:, 1:2], func=AF.Sqrt,
                                                 bias=eps_sb, scale=1.0)
                        nc.vector.reciprocal(out=rstd, in_=rstd)
                        nc.vector.tensor_scalar(out=xt[:, tt, :], in0=xt[:, tt, :],
                                                scalar1=mv[:, 0:1], scalar2=rstd,
                                                op0=OP.subtract, op1=OP.mult)
                    nc.sync.dma_start(out=xnr[:, ch * 8:(ch + 1) * 8, :], in_=xt)

            with tc.tile_pool(name="xnt", bufs=1) as xntp:
                xnT = []
                for kc in range(4):
                    t_ = xntp.tile([128, N], BF16, tag=f"xnT{kc}")
                    for ch in range(4):
                        nc.sync.dma_start_transpose(
                            t_[:, ch * 1024:(ch + 1) * 1024],
                            xn_dram[ch * 1024:(ch + 1) * 1024, kc * 128:(kc + 1) * 128])
                    xnT.append(t_)
                ones_row = xntp.tile([1, N], BF16, tag="ones")
                nc.sync.dma_start(out=ones_row, in_=ones_in[:, :])

                # ---------------- P3-P4: QKV^T GEMM for q/k cols + RoPE + assembly
                with tc.tile_pool(name="rope", bufs=1) as rp, \
                     tc.tile_pool(name="qkps", bufs=3, space="PSUM") as qkps:
                    cos_sb = rp.tile([128, N], BF16, tag="cos")
                    sin_sb = rp.tile([128, N], BF16, tag="sin")
                    nc.sync.dma_start(out=cos_sb, in_=cos_in[:, :])
                    nc.sync.dma_start(out=sin_sb, in_=sin_in[:, :])
                    A = rp.tile([128, N], BF16, tag="A")
                    Bt = rp.tile([128, N], BF16, tag="B")
                    # stream per 512-wide n-chunk: GEMM(A), GEMM(B), RoPE, assembly
                    for nc8 in range(QC):
                        sl = slice(nc8 * 512, (nc8 + 1) * 512)
                        for mi, dst in ((0, A), (1, Bt)):
                            psq = qkps.tile([128, 512], F32, tag="qk")
                            for kc in range(4):
                                nc.tensor.matmul(psq, w_sb[:, kc, mi * 128:(mi + 1) * 128],
                                                 xnT[kc][:, sl],
                                                 start=(kc == 0), stop=False)
                            nc.tensor.matmul(psq, w_sb[0:1, 4, mi * 128:(mi + 1) * 128],
                                             ones_row[0:1, sl], start=False, stop=True)
                            if mi == 0:
                                nc.vector.tensor_copy(out=dst[:, sl], in_=psq)
                            else:
                                nc.scalar.copy(out=dst[:, sl], in_=psq)
                        # RoPE: rotA = A*cos - B*sin ; rotB = B*cos + A*sin (overwrite A/B)
                        t1 = rp.tile([128, 512], BF16, tag="t1")
                        t2 = rp.tile([128, 512], BF16, tag="t2")
                        t3 = rp.tile([128, 512], BF16, tag="t3")
                        t4 = rp.tile([128, 512], BF16, tag="t4")
                        nc.vector.tensor_tensor(out=t1, in0=A[:, sl], in1=cos_sb[:, sl], op=OP.mult)
                        nc.vector.tensor_tensor(out=t2, in0=Bt[:, sl], in1=sin_sb[:, sl], op=OP.mult)
                        nc.vector.tensor_tensor(out=t3, in0=Bt[:, sl], in1=cos_sb[:, sl], op=OP.mult)
                        nc.vector.tensor_tensor(out=t4, in0=A[:, sl], in1=sin_sb[:, sl], op=OP.mult)
                        nc.vector.tensor_tensor(out=A[:, sl], in0=t1, in1=t2, op=OP.subtract)
                        nc.vector.tensor_tensor(out=Bt[:, sl], in0=t3, in1=t4, op=OP.add)
                        # assemble QT/KT (dim order per head: [first32, second32])
                        nc.gpsimd.tensor_copy(out=QT[0:32, sl], in_=A[0:32, sl])
                        nc.vector.tensor_copy(out=QT[32:64, sl], in_=Bt[0:32, sl])
                        nc.gpsimd.tensor_copy(out=QT[64:96, sl], in_=A[32:64, sl])
                        nc.vector.tensor_copy(out=QT[96:128, sl], in_=Bt[32:64, sl])
                        nc.gpsimd.tensor_copy(out=KT[0:32, sl], in_=A[64:96, sl])
                        nc.vector.tensor_copy(out=KT[32:64, sl], in_=Bt[64:96, sl])
                        nc.gpsimd.tensor_copy(out=KT[64:96, sl], in_=A[96:128, sl])
                        nc.vector.tensor_copy(out=KT[96:128, sl], in_=Bt[96:128, sl])

                # ---------------- P5: V natural GEMM (+ mask fold + ones cols)
                msk3 = msk_sb[:, :].rearrange("p (c one) -> p c one", one=1)
                nc.vector.tensor_copy(out=Vnat[:, 0:NBLK, 64:65], in_=msk3)
                nc.vector.tensor_copy(out=Vnat[:, 0:NBLK, 129:130], in_=msk3)
                with tc.tile_pool(name="vps", bufs=3, space="PSUM") as vps:
                    for nb in range(NBLK):
                        psv = vps.tile([128, 128], F32, tag="v")
                        for kc in range(4):
                            nc.tensor.matmul(psv, xnT[kc][:, nb * 128:(nb + 1) * 128],
                                             w_sb[:, kc, 256:384],
                                             start=(kc == 0), stop=False)
                        nc.tensor.matmul(psv, ones_row[0:1, nb * 128:(nb + 1) * 128],
                                         w_sb[0:1, 4, 256:384], start=False, stop=True)
                        dst = Vnat[:, nb, :].rearrange("p (g c) -> p g c", c=65)[:, :, 0:64]
                        src = psv.rearrange("p (g c) -> p g c", c=64)
                        nc.vector.tensor_scalar(out=dst, in0=src,
                                                scalar1=msk_sb[:, nb:nb + 1], scalar2=None,
                                                op0=OP.mult, op1=OP.bypass)

            # ---------------- P6: attention
            opks = []
            with (
                tc.tile_pool(name="spool", bufs=2, space="PSUM") as spool,
                tc.tile_pool(name="opool", bufs=2, space="PSUM") as opool,
                tc.tile_pool(name="ppool", bufs=3) as ppool,
                tc.tile_pool(name="rpool", bufs=4) as rpool,
                tc.tile_pool(name="rbpool", bufs=2) as rbpool,
                tc.tile_pool(name="rdram", bufs=4, space="DRAM") as rdram,
            ):
                for qc in range(QC):
                    qsl = slice(qc * 512, (qc + 1) * 512)
                    opk_pair = []
                    for h in range(2):
                        hsl = slice(h * 64, (h + 1) * 64)
                        qh = QT[hsl, qsl]
                        o_ps = opool.tile([65, 512], F32, tag="o")
                        for g in range(NGRP):
                            sgrp = spool.tile([128, 1536], F32, tag="s")
                            for j in range(3):
                                mc = 3 * g + j
                                js = slice(j * 512, (j + 1) * 512)
                                if mc < 32:
                                    nc.tensor.matmul(sgrp[:, js],
                                                     KT[hsl, mc * 128:(mc + 1) * 128],
                                                     qh, start=True, stop=True)
                                else:
                                    nc.tensor.matmul(sgrp[0:NPM, js],
                                                     KT[hsl, N:M_TOT],
                                                     qh, start=True, stop=True)
                            pgrp = ppool.tile([128, 1536], BF16, tag="p")
                            # last group's pm slice has 112 never-written psum rows;
                            # exp of stale-but-finite logits there is never read
                            # (AV contracts only rows 0:16 of the pm slice).
                            nc.scalar.activation(out=pgrp, in_=sgrp, func=AF.Exp, scale=SCALE)
                            for j in range(3):
                                mc = 3 * g + j
                                js = slice(j * 512, (j + 1) * 512)
                                if mc < 32:
                                    nc.tensor.matmul(o_ps, Vnat[:, mc, 65 * h:65 * h + 65],
                                                     pgrp[:, js],
                                                     start=(mc == 0), stop=(mc == 32))
                                else:
                                    nc.tensor.matmul(o_ps, Vnat[0:NPM, mc, 65 * h:65 * h + 65],
                                                     pgrp[0:NPM, js],
                                                     start=False, stop=True)
                        # normalize: r = 1/denom ; broadcast via DRAM bounce ; opk_h = numer * r
                        r_sb = rpool.tile([1, 512], F32, tag="r")
                        nc.vector.reciprocal(out=r_sb, in_=o_ps[64:65, :])
                        r_dr = rdram.tile([1, 512], F32, tag="rd")
                        nc.sync.dma_start(out=r_dr[:, :], in_=r_sb)
                        rd_ap = r_dr[:, :]
                        r_bc = bass.AP(tensor=rd_ap.tensor, offset=rd_ap.offset,
                                       ap=[[0, 64]] + list(rd_ap.ap[1:]))
                        r64 = rbpool.tile([64, 512], F32, tag="rb")
                        nc.gpsimd.dma_start(out=r64, in_=r_bc)
                        opk_h = okpool.tile([64, 512], BF16, tag="ok")
                        nc.vector.tensor_tensor(out=opk_h, in0=o_ps[0:64, :], in1=r64,
                                                op=OP.mult)
                        opk_pair.append(opk_h)
                    opks.append(opk_pair)

            # ---------------- P7: output projection (K split per head)
            with (
                tc.tile_pool(name="fps", bufs=4, space="PSUM") as fps,
                tc.tile_pool(name="fsb", bufs=8) as fsb,
            ):
                for qc in range(QC):
                    for mb in range(4):
                        psf = fps.tile([128, 512], F32, tag="f")
                        nc.tensor.matmul(psf, opks[qc][0][:, mb * 128:(mb + 1) * 128],
                                         wo0_sb, start=True, stop=False)
                        nc.tensor.matmul(psf, opks[qc][1][:, mb * 128:(mb + 1) * 128],
                                         wo1_sb, start=False, stop=True)
                        f_sb = fsb.tile([128, 512], F32, tag="fs")
                        if (qc * 4 + mb) % 2 == 0:
                            nc.scalar.copy(out=f_sb, in_=psf)
                        else:
                            nc.vector.tensor_copy(out=f_sb, in_=psf)
                        row0 = qc * 512 + mb * 128
                        eng = nc.sync if mb % 2 == 0 else nc.gpsimd
                        eng.dma_start(out=out_p[row0:row0 + 128, :], in_=f_sb)

    _split_excess_waits(nc)
    return nc


_STATE = {}


def _get_nc():
    if "nc" not in _STATE:
        _STATE["nc"] = build()
    return _STATE["nc"]


def _rope_tables():
    inv = 1.0 / (BASE ** (np.arange(0, DH, 2, dtype=np.float64) / DH))  # [32]
    pos = np.arange(N, dtype=np.float64)
    fr = pos[None, :] * inv[:, None]                                   # [32, N]
    cos32 = np.cos(fr)
    sin32 = np.sin(fr)
    cosf = np.tile(cos32, (4, 1)).astype(ml_dtypes.bfloat16)
    sinf = np.tile(sin32, (4, 1)).astype(ml_dtypes.bfloat16)
    return cosf, sinf


def kernel(x, mask, ln_w, ln_b, w_qkv, w_out, pm):
    bf = ml_dtypes.bfloat16
    f = np.float32
    x = np.asarray(x, f)
    mask_b = np.asarray(mask).astype(bool)
    ln_w = np.asarray(ln_w, f)
    ln_b = np.asarray(ln_b, f)
    w_qkv = np.asarray(w_qkv, f)
    w_out = np.asarray(w_out, f)
    pm = np.asarray(pm, f)

    cosf, sinf = _rope_tables()
    w_eff = ln_w[:, None] * w_qkv                     # [512, 1536]
    brow = ln_b @ w_qkv                               # [1536]

    in_maps = []
    for c in range(NCORES):
        bc = c // 4
        g0, g1 = (c % 4) * 2, (c % 4) * 2 + 1
        qcols = lambda g, lo, hi: np.arange(g * 64 + lo, g * 64 + hi)
        acols = np.concatenate([qcols(g0, 0, 32), qcols(g1, 0, 32),
                                512 + qcols(g0, 0, 32), 512 + qcols(g1, 0, 32)])
        bcols = np.concatenate([qcols(g0, 32, 64), qcols(g1, 32, 64),
                                512 + qcols(g0, 32, 64), 512 + qcols(g1, 32, 64)])
        vcols = np.concatenate([1024 + qcols(g0, 0, 64), 1024 + qcols(g1, 0, 64)])
        cols = np.concatenate([acols, bcols, vcols])  # [384]
        wq = np.zeros((5 * 128, 384), f)
        wq[0:512] = w_eff[:, cols]
        wq[512] = brow[cols]
        # pm[0, g] : [NPM, 64] -> K^T rows = dims, cols = pm idx
        pmk = np.concatenate([pm[0, g0].T, pm[0, g1].T], axis=0)  # [128, NPM]
        pmv = np.zeros((NPM, 130), f)
        pmv[:, 0:64] = pm[1, g0]
        pmv[:, 64] = 1.0
        pmv[:, 65:129] = pm[1, g1]
        pmv[:, 129] = 1.0
        mk = mask_b[bc, 0].astype(f).reshape(NBLK, 128).T  # [128, NBLK]
        in_maps.append(dict(
            x_in=x[bc].astype(bf),
            wq_in=wq.astype(bf),
            wo0_in=w_out[g0 * 64:(g0 + 1) * 64].astype(bf),
            wo1_in=w_out[g1 * 64:(g1 + 1) * 64].astype(bf),
            cos_in=cosf, sin_in=sinf,
            pmk_in=pmk.astype(bf), pmv_in=pmv.astype(bf),
            msk_in=np.ascontiguousarray(mk),
            ones_in=np.ones((1, N), bf),
        ))

    global _LAST_IN_MAPS
    _LAST_IN_MAPS = in_maps
    nc = _get_nc()
    res = run_bass_kernel_spmd(nc, in_maps, core_ids=list(range(NCORES)))
    out = np.zeros((B, N, D), f)
    for c in range(NCORES):
        out[c // 4] += res.results[c]["out_p"]
    return out


# revision 36
# speedup vs baseline: 339.4400x; 1.0131x over previous
"""Trainium2 Bass kernel for nn_BidirectionalAttention (LayerNorm -> QKV -> RoPE ->
attention with 16 persistent-memory KV tokens -> out projection).

Sharding: 8 cores = (batch b=2) x (4 head-pairs). Each core computes its batch's
LayerNorm + QKV for its 2 heads, full attention over n=4096 (+16 pm) keys, and a
partial output projection; the host sums the 4 partials per batch.

Self-contained: hardcodes all shapes from the problem spec.
"""
import sys

sys.path.insert(0, "/opt/trn_rl_repo")

import numpy as np
import ml_dtypes

import concourse.bass as bass
import concourse.tile as tile
from concourse import mybir
from concourse.bass_utils import run_bass_kernel_spmd

BF16 = mybir.dt.bfloat16
F32 = mybir.dt.float32
AF = mybir.ActivationFunctionType
OP = mybir.AluOpType

B, N, D = 2, 4096, 512
H, DH, NPM = 8, 64, 16
BASE, EPS = 10000.0, 1e-5
SCALE = DH ** -0.5
NCORES = 8
M_TOT = N + NPM          # 4112 keys
MCHUNKS = 33             # 32 seq chunks of 128 + 1 pm chunk of 16
NGRP = 11                # exp/AV groups of 3 m-chunks
QC = 8                   # query chunks of 512
NBLK = 32                # n blocks of 128


def _split_excess_waits(nc, max_waits=1):
    """walrus in this container rejects >1 sync waits per instruction; hoist
    extras onto same-engine nops inserted just before (same sequencer order)."""
    cnt = 0
    for fn in nc.m.functions:
        for bb in fn.blocks:
            insts = bb.instructions
            i = 0
            while i < len(insts):
                inst = insts[i]
                si = inst.sync_info
                if si is not None and si.on_wait is not None and len(si.on_wait) > max_waits:
                    waits = list(si.on_wait)
                    extra, keep = waits[:-max_waits], waits[-max_waits:]
                    nops = []
                    for j in range(0, len(extra), max_waits):
                        cnt += 1
                        nop = mybir.InstNoOp(name=f"I-waitsplit-{cnt}-{inst.name}",
                                             engine=inst.engine, ins=[], outs=[])
                        nop.sync_info = mybir.SyncInfo(on_wait=extra[j:j + max_waits],
                                                       on_update=[])
                        nc.register_instruction(nop, overwrite=True)
                        nops.append(nop)
                    si.on_wait = keep
                    for k, nop in enumerate(nops):
                        insts.insert(i + k, nop)
                    i += len(nops)
                i += 1
    return cnt


def build(reps=1):
    nc = bass.Bass()

    x_in = nc.dram_tensor("x_in", [N, D], BF16, kind="ExternalInput")
    wq_in = nc.dram_tensor("wq_in", [5 * 128, 384], BF16, kind="ExternalInput")
    wo0_in = nc.dram_tensor("wo0_in", [64, 512], BF16, kind="ExternalInput")
    wo1_in = nc.dram_tensor("wo1_in", [64, 512], BF16, kind="ExternalInput")
    cos_in = nc.dram_tensor("cos_in", [128, N], BF16, kind="ExternalInput")
    sin_in = nc.dram_tensor("sin_in", [128, N], BF16, kind="ExternalInput")
    pmk_in = nc.dram_tensor("pmk_in", [128, NPM], BF16, kind="ExternalInput")
    pmv_in = nc.dram_tensor("pmv_in", [NPM, 130], BF16, kind="ExternalInput")
    msk_in = nc.dram_tensor("msk_in", [128, NBLK], F32, kind="ExternalInput")
    ones_in = nc.dram_tensor("ones_in", [1, N], BF16, kind="ExternalInput")
    out_p = nc.dram_tensor("out_p", [N, D], F32, kind="ExternalOutput")

    import contextlib
    with tile.TileContext(nc) as tc:
      with (tc.For_i(0, reps, 1) if reps > 1 else contextlib.nullcontext()):
        with (
            tc.tile_pool(name="persist", bufs=1) as pers,
            tc.tile_pool(name="okpool", bufs=16) as okpool,
            tc.tile_pool(name="dram", bufs=1, space="DRAM") as dr,
        ):
            w_sb = pers.tile([128, 5, 384], BF16)
            nc.sync.dma_start(out=w_sb, in_=wq_in.rearrange("(kc p) m -> p kc m", p=128))
            wo0_sb = pers.tile([64, 512], BF16)
            wo1_sb = pers.tile([64, 512], BF16)
            nc.sync.dma_start(out=wo0_sb, in_=wo0_in[:, :])
            nc.sync.dma_start(out=wo1_sb, in_=wo1_in[:, :])
            msk_sb = pers.tile([128, NBLK], F32)
            nc.sync.dma_start(out=msk_sb, in_=msk_in[:, :])
            eps_sb = pers.tile([128, 1], F32)
            nc.vector.memset(eps_sb, EPS)
            QT = pers.tile([128, N], BF16)       # [q_g0(64); q_g1(64)] x n
            KT = pers.tile([128, M_TOT], BF16)   # [k_g0(64); k_g1(64)] x (n + pm)
            Vnat = pers.tile([128, MCHUNKS, 130], BF16)  # per m-chunk: [v_g0(64), m, v_g1(64), m]
            nc.sync.dma_start(out=KT[:, N:M_TOT], in_=pmk_in[:, :])
            nc.sync.dma_start(out=Vnat[0:NPM, 32, :], in_=pmv_in[:, :])
            xn_dram = dr.tile([N, D], BF16)

            # ---------------- P0-P2: load x, LayerNorm (in place), DRAM roundtrip transpose
            # chunked by groups of 8 n-tiles; transposes are interleaved with the
            # stores in trace order (Tile's xbar-mode guard serializes XPOSE
            # against DMACopies in trace order, so late transposes would stall)
            with tc.tile_pool(name="xnt", bufs=1) as xntp:
              xnT = [xntp.tile([128, N], BF16, tag=f"xnT{kc}", name=f"xnT{kc}")
                     for kc in range(4)]
              ones_row = xntp.tile([1, N], BF16, tag="ones")
              nc.sync.dma_start(out=ones_row, in_=ones_in[:, :])
              with tc.tile_pool(name="xpool", bufs=1) as xp, tc.tile_pool(name="lnp", bufs=4) as lnp:
                xr = x_in.rearrange("(t p) d -> p t d", p=128)
                xnr = xn_dram.rearrange("(t p) d -> p t d", p=128)
                xch = []
                for ch in range(4):
                    xt = xp.tile([128, 8, D], BF16, tag=f"x{ch}")
                    nc.sync.dma_start(out=xt, in_=xr[:, ch * 8:(ch + 1) * 8, :])
                    xch.append(xt)
                scr = xp.tile([128, D], BF16, tag="scr")  # ACT-stats discard buffer
                inv_d = 1.0 / D
                for ch in range(4):
                    xt = xch[ch]
                    for tt in range(8):
                        mv = lnp.tile([128, 2], F32, tag="mv")
                        rstd = lnp.tile([128, 1], F32, tag="rstd")
                        if tt % 2 == 0:  # split LN stats between DVE and ACT
                            # DVE stats path
                            stats = lnp.tile([128, 6], F32, tag="stats")
                            nc.vector.bn_stats(out=stats, in_=xt[:, tt, :])
                            nc.vector.bn_aggr(out=mv, in_=stats)
                            nc.scalar.activation(out=rstd, in_=mv[:, 1:2], func=AF.Sqrt,
                                                 bias=eps_sb, scale=1.0)
                        else:
                            # ACT stats path: sum via Copy+accum, sumsq via Square+accum
                            sums = lnp.tile([128, 2], F32, tag="sums")
                            nc.scalar.activation(out=scr, in_=xt[:, tt, :], func=AF.Copy,
                                                 accum_out=sums[:, 0:1])
                            nc.scalar.activation(out=scr, in_=xt[:, tt, :], func=AF.Square,
                                                 accum_out=sums[:, 1:2])
                            nc.vector.tensor_scalar(out=mv, in0=sums, scalar1=inv_d,
                                                    scalar2=None, op0=OP.mult, op1=OP.bypass)
                            mu2 = lnp.tile([128, 1], F32, tag="mu2")
                            nc.vector.tensor_tensor(out=mu2, in0=mv[:, 0:1], in1=mv[:, 0:1],
                                                    op=OP.mult)
                            # var = meansq - mu^2 ; rstd = 1/sqrt(var+eps)
                            nc.vector.tensor_tensor(out=mv[:, 1:2], in0=mv[:, 1:2], in1=mu2,
                                                    op=OP.subtract)
                            nc.scalar.activation(out=rstd, in_=mv[:, 1:2], func=AF.Sqrt,
                                                 bias=eps_sb, scale=1.0)
                        nc.vector.reciprocal(out=rstd, in_=rstd)
                        nc.vector.tensor_scalar(out=xt[:, tt, :], in0=xt[:, tt, :],
                                                scalar1=mv[:, 0:1], scalar2=rstd,
                                                op0=OP.subtract, op1=OP.mult)
                    nc.sync.dma_start(out=xnr[:, ch * 8:(ch + 1) * 8, :], in_=xt)
                    for kc in range(4):
                        nc.sync.dma_start_transpose(
                            xnT[kc][:, ch * 1024:(ch + 1) * 1024],
                            xn_dram[ch * 1024:(ch + 1) * 1024, kc * 128:(kc + 1) * 128])

              if True:
                # ---------------- P3-P5: QKV^T GEMM + RoPE + assembly + Vnat, streamed
                msk3 = msk_sb[:, :].rearrange("p (c one) -> p c one", one=1)
                nc.vector.tensor_copy(out=Vnat[:, 0:NBLK, 64:65], in_=msk3)
                nc.vector.tensor_copy(out=Vnat[:, 0:NBLK, 129:130], in_=msk3)
                with tc.tile_pool(name="rope", bufs=1) as rp, \
                     tc.tile_pool(name="qkps", bufs=3, space="PSUM") as qkps, \
                     tc.tile_pool(name="vps", bufs=3, space="PSUM") as vps:
                    cos_sb = rp.tile([128, N], BF16, tag="cos")
                    sin_sb = rp.tile([128, N], BF16, tag="sin")
                    nc.sync.dma_start(out=cos_sb, in_=cos_in[:, :])
                    nc.sync.dma_start(out=sin_sb, in_=sin_in[:, :])
                    # stream per 512-wide n-chunk: GEMM(A), GEMM(B), RoPE, assembly
                    # (per-chunk tiles so chunks pipeline independently)
                    for nc8 in range(QC):
                        sl = slice(nc8 * 512, (nc8 + 1) * 512)
                        ab = []
                        for mi in range(2):
                            psq = qkps.tile([128, 512], F32, tag="qk")
                            for kc in range(4):
                                nc.tensor.matmul(psq, w_sb[:, kc, mi * 128:(mi + 1) * 128],
                                                 xnT[kc][:, sl],
                                                 start=(kc == 0), stop=False)
                            nc.tensor.matmul(psq, w_sb[0:1, 4, mi * 128:(mi + 1) * 128],
                                             ones_row[0:1, sl], start=False, stop=True)
                            dst = rp.tile([128, 512], BF16, tag=f"ab{mi}", bufs=3,
                                          name=f"ab{mi}_{nc8}")
                            if mi == 0:
                                nc.vector.tensor_copy(out=dst, in_=psq)
                            else:
                                nc.scalar.copy(out=dst, in_=psq)
                            ab.append(dst)
                        A, Bt = ab
                        # RoPE: rotA = A*cos - B*sin ; rotB = B*cos + A*sin
                        t1 = rp.tile([128, 512], BF16, tag="t1", bufs=3)
                        t2 = rp.tile([128, 512], BF16, tag="t2", bufs=3)
                        t3 = rp.tile([128, 512], BF16, tag="t3", bufs=3)
                        t4 = rp.tile([128, 512], BF16, tag="t4", bufs=3)
                        ra = rp.tile([128, 512], BF16, tag="ra", bufs=3)
                        rb = rp.tile([128, 512], BF16, tag="rb", bufs=3)
                        nc.vector.tensor_tensor(out=t1, in0=A, in1=cos_sb[:, sl], op=OP.mult)
                        nc.vector.tensor_tensor(out=t2, in0=Bt, in1=sin_sb[:, sl], op=OP.mult)
                        nc.vector.tensor_tensor(out=t3, in0=Bt, in1=cos_sb[:, sl], op=OP.mult)
                        nc.vector.tensor_tensor(out=t4, in0=A, in1=sin_sb[:, sl], op=OP.mult)
                        nc.vector.tensor_tensor(out=ra, in0=t1, in1=t2, op=OP.subtract)
                        nc.vector.tensor_tensor(out=rb, in0=t3, in1=t4, op=OP.add)
                        # assemble QT/KT (dim order per head: [first32, second32]);
                        # KT gates all of attention -> fast DVE copies; QT is only
                        # needed per-qc (slack) -> slower GPSIMD copies are fine
                        nc.gpsimd.tensor_copy(out=QT[0:32, sl], in_=ra[0:32, :])
                        nc.gpsimd.tensor_copy(out=QT[32:64, sl], in_=rb[0:32, :])
                        nc.gpsimd.tensor_copy(out=QT[64:96, sl], in_=ra[32:64, :])
                        nc.gpsimd.tensor_copy(out=QT[96:128, sl], in_=rb[32:64, :])
                        nc.vector.tensor_copy(out=KT[0:32, sl], in_=ra[64:96, :])
                        nc.vector.tensor_copy(out=KT[32:64, sl], in_=rb[64:96, :])
                        nc.vector.tensor_copy(out=KT[64:96, sl], in_=ra[96:128, :])
                        nc.vector.tensor_copy(out=KT[96:128, sl], in_=rb[96:128, :])
                        # V natural GEMM for this n-range (+ mask fold)
                        for nb in range(nc8 * 4, nc8 * 4 + 4):
                            psv = vps.tile([128, 128], F32, tag="v")
                            for kc in range(4):
                                nc.tensor.matmul(psv, xnT[kc][:, nb * 128:(nb + 1) * 128],
                                                 w_sb[:, kc, 256:384],
                                                 start=(kc == 0), stop=False)
                            nc.tensor.matmul(psv, ones_row[0:1, nb * 128:(nb + 1) * 128],
                                             w_sb[0:1, 4, 256:384], start=False, stop=True)
                            vdst = Vnat[:, nb, :].rearrange("p (g c) -> p g c", c=65)[:, :, 0:64]
                            vsrc = psv.rearrange("p (g c) -> p g c", c=64)
                            nc.vector.tensor_scalar(out=vdst, in0=vsrc,
                                                    scalar1=msk_sb[:, nb:nb + 1], scalar2=None,
                                                    op0=OP.mult, op1=OP.bypass)

            # ---------------- P6: attention
            opks = []
            with (
                tc.tile_pool(name="spool", bufs=2, space="PSUM") as spool,
                tc.tile_pool(name="opool", bufs=2, space="PSUM") as opool,
                tc.tile_pool(name="ppool", bufs=3) as ppool,
                tc.tile_pool(name="rpool", bufs=4) as rpool,
                tc.tile_pool(name="rbpool", bufs=2) as rbpool,
                tc.tile_pool(name="rdram", bufs=4, space="DRAM") as rdram,
            ):
                for qc in range(QC):
                    qsl = slice(qc * 512, (qc + 1) * 512)
                    opk_pair = []
                    for h in range(2):
                        hsl = slice(h * 64, (h + 1) * 64)
                        qh = QT[hsl, qsl]
                        o_ps = opool.tile([65, 512], F32, tag="o")
                        for g in range(NGRP):
                            sgrp = spool.tile([128, 1536], F32, tag="s")
                            for j in range(3):
                                mc = 3 * g + j
                                js = slice(j * 512, (j + 1) * 512)
                                if mc < 32:
                                    nc.tensor.matmul(sgrp[:, js],
                                                     KT[hsl, mc * 128:(mc + 1) * 128],
                                                     qh, start=True, stop=True)
                                else:
                                    nc.tensor.matmul(sgrp[0:NPM, js],
                                                     KT[hsl, N:M_TOT],
                                                     qh, start=True, stop=True)
                            pgrp = ppool.tile([128, 1536], BF16, tag="p")
                            # last group's pm slice has 112 never-written psum rows;
                            # exp of stale-but-finite logits there is never read
                            # (AV contracts only rows 0:16 of the pm slice).
                            nc.scalar.activation(out=pgrp, in_=sgrp, func=AF.Exp, scale=SCALE)
                            for j in range(3):
                                mc = 3 * g + j
                                js = slice(j * 512, (j + 1) * 512)
                                if mc < 32:
                                    nc.tensor.matmul(o_ps, Vnat[:, mc, 65 * h:65 * h + 65],
                                                     pgrp[:, js],
                                                     start=(mc == 0), stop=(mc == 32))
                                else:
                                    nc.tensor.matmul(o_ps, Vnat[0:NPM, mc, 65 * h:65 * h + 65],
                                                     pgrp[0:NPM, js],
                                                     start=False, stop=True)
                        # normalize: r = 1/denom ; broadcast via DRAM bounce ; opk_h = numer * r
                        r_sb = rpool.tile([1, 512], F32, tag="r")
                        nc.vector.reciprocal(out=r_sb, in_=o_ps[64:65, :])
                        r_dr = rdram.tile([1, 512], F32, tag="rd")
                        nc.sync.dma_start(out=r_dr[:, :], in_=r_sb)
                        rd_ap = r_dr[:, :]
                        r_bc = bass.AP(tensor=rd_ap.tensor, offset=rd_ap.offset,
                                       ap=[[0, 64]] + list(rd_ap.ap[1:]))
                        r64 = rbpool.tile([64, 512], F32, tag="rb")
                        nc.gpsimd.dma_start(out=r64, in_=r_bc)
                        opk_h = okpool.tile([64, 512], BF16, tag="ok")
                        nc.vector.tensor_tensor(out=opk_h, in0=o_ps[0:64, :], in1=r64,
                                                op=OP.mult)
                        opk_pair.append(opk_h)
                    opks.append(opk_pair)

            # ---------------- P7: output projection (K split per head)
            with (
                tc.tile_pool(name="fps", bufs=4, space="PSUM") as fps,
                tc.tile_pool(name="fsb", bufs=8) as fsb,
            ):
                for qc in range(QC):
                    for mb in range(4):
                        psf = fps.tile([128, 512], F32, tag="f")
                        nc.tensor.matmul(psf, opks[qc][0][:, mb * 128:(mb + 1) * 128],
                                         wo0_sb, start=True, stop=False)
                        nc.tensor.matmul(psf, opks[qc][1][:, mb * 128:(mb + 1) * 128],
                                         wo1_sb, start=False, stop=True)
                        f_sb = fsb.tile([128, 512], F32, tag="fs")
                        if (qc * 4 + mb) % 2 == 0:
                            nc.scalar.copy(out=f_sb, in_=psf)
                        else:
                            nc.vector.tensor_copy(out=f_sb, in_=psf)
                        row0 = qc * 512 + mb * 128
                        eng = nc.sync if mb % 2 == 0 else nc.gpsimd
                        eng.dma_start(out=out_p[row0:row0 + 128, :], in_=f_sb)

    _split_excess_waits(nc)
    return nc


_STATE = {}


def _get_nc():
    if "nc" not in _STATE:
        _STATE["nc"] = build()
    return _STATE["nc"]


def _rope_tables():
    inv = 1.0 / (BASE ** (np.arange(0, DH, 2, dtype=np.float64) / DH))  # [32]
    pos = np.arange(N, dtype=np.float64)
    fr = pos[None, :] * inv[:, None]                                   # [32, N]
    cos32 = np.cos(fr)
    sin32 = np.sin(fr)
    cosf = np.tile(cos32, (4, 1)).astype(ml_dtypes.bfloat16)
    sinf = np.tile(sin32, (4, 1)).astype(ml_dtypes.bfloat16)
    return cosf, sinf


def kernel(x, mask, ln_w, ln_b, w_qkv, w_out, pm):
    bf = ml_dtypes.bfloat16
    f = np.float32
    x = np.asarray(x, f)
    mask_b = np.asarray(mask).astype(bool)
    ln_w = np.asarray(ln_w, f)
    ln_b = np.asarray(ln_b, f)
    w_qkv = np.asarray(w_qkv, f)
    w_out = np.asarray(w_out, f)
    pm = np.asarray(pm, f)

    cosf, sinf = _rope_tables()
    w_eff = ln_w[:, None] * w_qkv                     # [512, 1536]
    brow = ln_b @ w_qkv                               # [1536]

    in_maps = []
    for c in range(NCORES):
        bc = c // 4
        g0, g1 = (c % 4) * 2, (c % 4) * 2 + 1
        qcols = lambda g, lo, hi: np.arange(g * 64 + lo, g * 64 + hi)
        acols = np.concatenate([qcols(g0, 0, 32), qcols(g1, 0, 32),
                                512 + qcols(g0, 0, 32), 512 + qcols(g1, 0, 32)])
        bcols = np.concatenate([qcols(g0, 32, 64), qcols(g1, 32, 64),
                                512 + qcols(g0, 32, 64), 512 + qcols(g1, 32, 64)])
        vcols = np.concatenate([1024 + qcols(g0, 0, 64), 1024 + qcols(g1, 0, 64)])
        cols = np.concatenate([acols, bcols, vcols])  # [384]
        wq = np.zeros((5 * 128, 384), f)
        wq[0:512] = w_eff[:, cols]
        wq[512] = brow[cols]
        # pm[0, g] : [NPM, 64] -> K^T rows = dims, cols = pm idx
        pmk = np.concatenate([pm[0, g0].T, pm[0, g1].T], axis=0)  # [128, NPM]
        pmv = np.zeros((NPM, 130), f)
        pmv[:, 0:64] = pm[1, g0]
        pmv[:, 64] = 1.0
        pmv[:, 65:129] = pm[1, g1]
        pmv[:, 129] = 1.0
        mk = mask_b[bc, 0].astype(f).reshape(NBLK, 128).T  # [128, NBLK]
        in_maps.append(dict(
            x_in=x[bc].astype(bf),
            wq_in=wq.astype(bf),
            wo0_in=w_out[g0 * 64:(g0 + 1) * 64].astype(bf),
            wo1_in=w_out[g1 * 64:(g1 + 1) * 64].astype(bf),
            cos_in=cosf, sin_in=sinf,
            pmk_in=pmk.astype(bf), pmv_in=pmv.astype(bf),
            msk_in=np.ascontiguousarray(mk),
            ones_in=np.ones((1, N), bf),
        ))

    global _LAST_IN_MAPS
    _LAST_IN_MAPS = in_maps
    nc = _get_nc()
    res = run_bass_kernel_spmd(nc, in_maps, core_ids=list(range(NCORES)))
    out = np.zeros((B, N, D), f)
    for c in range(NCORES):
        out[c // 4] += res.results[c]["out_p"]
    return out


# revision 63
# speedup vs baseline: 344.0946x; 1.0137x over previous
"""Trainium2 Bass kernel for nn_BidirectionalAttention (LayerNorm -> QKV -> RoPE ->
attention with 16 persistent-memory KV tokens -> out projection).

Sharding: 8 cores = (batch b=2) x (4 head-pairs). Each core computes its batch's
LayerNorm + QKV for its 2 heads, full attention over n=4096 (+16 pm) keys, and a
partial output projection; the host sums the 4 partials per batch.

Self-contained: hardcodes all shapes from the problem spec.
"""
import sys

sys.path.insert(0, "/opt/trn_rl_repo")

import numpy as np
import ml_dtypes

import concourse.bass as bass
import concourse.tile as tile
from concourse import mybir
from concourse.bass_utils import run_bass_kernel_spmd

BF16 = mybir.dt.bfloat16
F32 = mybir.dt.float32
AF = mybir.ActivationFunctionType
OP = mybir.AluOpType

B, N, D = 2, 4096, 512
H, DH, NPM = 8, 64, 16
BASE, EPS = 10000.0, 1e-5
SCALE = DH ** -0.5
NCORES = 8
M_TOT = N + NPM          # 4112 keys
MCHUNKS = 33             # 32 seq chunks of 128 + 1 pm chunk of 16
NGRP = 11                # exp/AV groups of 3 m-chunks
QC = 8                   # query chunks of 512
NBLK = 32                # n blocks of 128


def _split_excess_waits(nc, max_waits=1):
    """walrus in this container rejects >1 sync waits per instruction; hoist
    extras onto same-engine nops inserted just before (same sequencer order)."""
    cnt = 0
    for fn in nc.m.functions:
        for bb in fn.blocks:
            insts = bb.instructions
            i = 0
            while i < len(insts):
                inst = insts[i]
                si = inst.sync_info
                if si is not None and si.on_wait is not None and len(si.on_wait) > max_waits:
                    waits = list(si.on_wait)
                    extra, keep = waits[:-max_waits], waits[-max_waits:]
                    nops = []
                    for j in range(0, len(extra), max_waits):
                        cnt += 1
                        nop = mybir.InstNoOp(name=f"I-waitsplit-{cnt}-{inst.name}",
                                             engine=inst.engine, ins=[], outs=[])
                        nop.sync_info = mybir.SyncInfo(on_wait=extra[j:j + max_waits],
                                                       on_update=[])
                        nc.register_instruction(nop, overwrite=True)
                        nops.append(nop)
                    si.on_wait = keep
                    for k, nop in enumerate(nops):
                        insts.insert(i + k, nop)
                    i += len(nops)
                i += 1
    return cnt


def build(reps=1):
    nc = bass.Bass()

    x_in = nc.dram_tensor("x_in", [N, D], BF16, kind="ExternalInput")
    wq_in = nc.dram_tensor("wq_in", [5 * 128, 384], BF16, kind="ExternalInput")
    wo0_in = nc.dram_tensor("wo0_in", [64, 512], BF16, kind="ExternalInput")
    wo1_in = nc.dram_tensor("wo1_in", [64, 512], BF16, kind="ExternalInput")
    cos_in = nc.dram_tensor("cos_in", [128, N], BF16, kind="ExternalInput")
    sin_in = nc.dram_tensor("sin_in", [128, N], BF16, kind="ExternalInput")
    pmk_in = nc.dram_tensor("pmk_in", [128, NPM], BF16, kind="ExternalInput")
    pmv_in = nc.dram_tensor("pmv_in", [NPM, 130], BF16, kind="ExternalInput")
    msk_in = nc.dram_tensor("msk_in", [128, NBLK], F32, kind="ExternalInput")
    ones_in = nc.dram_tensor("ones_in", [1, N], BF16, kind="ExternalInput")
    out_p = nc.dram_tensor("out_p", [N, D], F32, kind="ExternalOutput")

    import contextlib
    with tile.TileContext(nc) as tc:
      with (tc.For_i(0, reps, 1) if reps > 1 else contextlib.nullcontext()):
        with (
            tc.tile_pool(name="persist", bufs=1) as pers,
            tc.tile_pool(name="okpool", bufs=16) as okpool,
            tc.tile_pool(name="dram", bufs=1, space="DRAM") as dr,
            # attention pools opened up-front: spool's banks have no preamble
            # tenants, so attention S/exp groups can start as soon as their
            # KT/QT chunks exist; preamble GEMM psum shares opool's slots.
            tc.tile_pool(name="spool", bufs=2, space="PSUM") as spool,
            tc.tile_pool(name="opool", bufs=2, space="PSUM") as opool,
            tc.tile_pool(name="ppool", bufs=16) as ppool,
            tc.tile_pool(name="rpool", bufs=4) as rpool,
            tc.tile_pool(name="rbpool", bufs=2) as rbpool,
            tc.tile_pool(name="fpool", bufs=8) as fpool,
            tc.tile_pool(name="rdram", bufs=4, space="DRAM") as rdram,
        ):
            w_sb = pers.tile([128, 5, 384], BF16)
            nc.sync.dma_start(out=w_sb, in_=wq_in.rearrange("(kc p) m -> p kc m", p=128))
            wo_sb = pers.tile([128, 512], BF16)
            nc.sync.dma_start(out=wo_sb[0:64, :], in_=wo0_in[:, :])
            nc.sync.dma_start(out=wo_sb[64:128, :], in_=wo1_in[:, :])
            msk_sb = pers.tile([128, NBLK], F32)
            nc.sync.dma_start(out=msk_sb, in_=msk_in[:, :])
            eps_sb = pers.tile([128, 1], F32)
            nc.vector.memset(eps_sb, EPS)
            QT = pers.tile([128, N], BF16)       # [q_g0(64); q_g1(64)] x n
            KT = pers.tile([128, M_TOT], BF16)   # [k_g0(64); k_g1(64)] x (n + pm)
            Vnat = pers.tile([128, MCHUNKS, 130], BF16)  # per m-chunk: [v_g0(64), m, v_g1(64), m]
            nc.sync.dma_start(out=KT[:, N:M_TOT], in_=pmk_in[:, :])
            nc.sync.dma_start(out=Vnat[0:NPM, 32, :], in_=pmv_in[:, :])
            xn_dram = dr.tile([N, D], BF16)

            # ---------------- P0-P2: load x, LayerNorm (in place), DRAM roundtrip transpose
            # chunked by groups of 8 n-tiles; transposes are interleaved with the
            # stores in trace order (Tile's xbar-mode guard serializes XPOSE
            # against DMACopies in trace order, so late transposes would stall)
            with tc.tile_pool(name="xnt", bufs=1) as xntp:
              xnT = [xntp.tile([128, N], BF16, tag=f"xnT{kc}", name=f"xnT{kc}")
                     for kc in range(4)]
              ones_row = xntp.tile([1, N], BF16, tag="ones")
              nc.sync.dma_start(out=ones_row, in_=ones_in[:, :])
              with tc.tile_pool(name="xpool", bufs=1) as xp, tc.tile_pool(name="lnp", bufs=4) as lnp:
                xr = x_in.rearrange("(t p) d -> p t d", p=128)
                xnr = xn_dram.rearrange("(t p) d -> p t d", p=128)
                xch = []
                for ch in range(4):
                    xt = xp.tile([128, 8, D], BF16, tag=f"x{ch}")
                    nc.sync.dma_start(out=xt, in_=xr[:, ch * 8:(ch + 1) * 8, :])
                    xch.append(xt)
                scr = xp.tile([128, D], BF16, tag="scr")  # ACT-stats discard buffer
                inv_d = 1.0 / D
                for ch in range(4):
                    xt = xch[ch]
                    for tt in range(8):
                        mv = lnp.tile([128, 2], F32, tag="mv")
                        rstd = lnp.tile([128, 1], F32, tag="rstd")
                        if tt % 2 == 0:  # split LN stats between DVE and ACT
                            # DVE stats path
                            stats = lnp.tile([128, 6], F32, tag="stats")
                            nc.vector.bn_stats(out=stats, in_=xt[:, tt, :])
                            nc.vector.bn_aggr(out=mv, in_=stats)
                            nc.scalar.activation(out=rstd, in_=mv[:, 1:2], func=AF.Sqrt,
                                                 bias=eps_sb, scale=1.0)
                        else:
                            # ACT stats path: sum via Copy+accum, sumsq via Square+accum
                            sums = lnp.tile([128, 2], F32, tag="sums")
                            nc.scalar.activation(out=scr, in_=xt[:, tt, :], func=AF.Copy,
                                                 accum_out=sums[:, 0:1])
                            nc.scalar.activation(out=scr, in_=xt[:, tt, :], func=AF.Square,
                                                 accum_out=sums[:, 1:2])
                            nc.vector.tensor_scalar(out=mv, in0=sums, scalar1=inv_d,
                                                    scalar2=None, op0=OP.mult, op1=OP.bypass)
                            mu2 = lnp.tile([128, 1], F32, tag="mu2")
                            nc.vector.tensor_tensor(out=mu2, in0=mv[:, 0:1], in1=mv[:, 0:1],
                                                    op=OP.mult)
                            # var = meansq - mu^2 ; rstd = 1/sqrt(var+eps)
                            nc.vector.tensor_tensor(out=mv[:, 1:2], in0=mv[:, 1:2], in1=mu2,
                                                    op=OP.subtract)
                            nc.scalar.activation(out=rstd, in_=mv[:, 1:2], func=AF.Sqrt,
                                                 bias=eps_sb, scale=1.0)
                        nc.vector.reciprocal(out=rstd, in_=rstd)
                        nc.vector.tensor_scalar(out=xt[:, tt, :], in0=xt[:, tt, :],
                                                scalar1=mv[:, 0:1], scalar2=rstd,
                                                op0=OP.subtract, op1=OP.mult)
                    nc.sync.dma_start(out=xnr[:, ch * 8:(ch + 1) * 8, :], in_=xt)
                    for kc in range(4):
                        nc.sync.dma_start_transpose(
                            xnT[kc][:, ch * 1024:(ch + 1) * 1024],
                            xn_dram[ch * 1024:(ch + 1) * 1024, kc * 128:(kc + 1) * 128])

              if True:
                # ---------------- P3-P5: QKV^T GEMM + RoPE + assembly + Vnat, streamed
                msk3 = msk_sb[:, :].rearrange("p (c one) -> p c one", one=1)
                nc.vector.tensor_copy(out=Vnat[:, 0:NBLK, 64:65], in_=msk3)
                nc.vector.tensor_copy(out=Vnat[:, 0:NBLK, 129:130], in_=msk3)
                with tc.tile_pool(name="rope", bufs=1) as rp:
                    cos_sb = rp.tile([128, N], BF16, tag="cos")
                    sin_sb = rp.tile([128, N], BF16, tag="sin")
                    nc.sync.dma_start(out=cos_sb, in_=cos_in[:, :])
                    nc.sync.dma_start(out=sin_sb, in_=sin_in[:, :])
                    # stream per 512-wide n-chunk: GEMM(A), GEMM(B), RoPE, assembly
                    # (per-chunk tiles so chunks pipeline independently)
                    for nc8 in range(QC):
                        sl = slice(nc8 * 512, (nc8 + 1) * 512)
                        ab = []
                        for mi in range(2):
                            psq = opool.tile([128, 512], F32, tag="o")
                            for kc in range(4):
                                nc.tensor.matmul(psq, w_sb[:, kc, mi * 128:(mi + 1) * 128],
                                                 xnT[kc][:, sl],
                                                 start=(kc == 0), stop=False)
                            nc.tensor.matmul(psq, w_sb[0:1, 4, mi * 128:(mi + 1) * 128],
                                             ones_row[0:1, sl], start=False, stop=True)
                            dst = rp.tile([128, 512], BF16, tag=f"ab{mi}", bufs=3,
                                          name=f"ab{mi}_{nc8}")
                            if mi == 0:
                                nc.vector.tensor_copy(out=dst, in_=psq)
                            else:
                                nc.scalar.copy(out=dst, in_=psq)
                            ab.append(dst)
                        A, Bt = ab
                        # RoPE: rotA = A*cos - B*sin ; rotB = B*cos + A*sin
                        t1 = rp.tile([128, 512], BF16, tag="t1", bufs=3)
                        t2 = rp.tile([128, 512], BF16, tag="t2", bufs=3)
                        t3 = rp.tile([128, 512], BF16, tag="t3", bufs=3)
                        t4 = rp.tile([128, 512], BF16, tag="t4", bufs=3)
                        ra = rp.tile([128, 512], BF16, tag="ra", bufs=3)
                        rb = rp.tile([128, 512], BF16, tag="rb", bufs=3)
                        nc.vector.tensor_tensor(out=t1, in0=A, in1=cos_sb[:, sl], op=OP.mult)
                        nc.vector.tensor_tensor(out=t2, in0=Bt, in1=sin_sb[:, sl], op=OP.mult)
                        nc.vector.tensor_tensor(out=t3, in0=Bt, in1=cos_sb[:, sl], op=OP.mult)
                        nc.vector.tensor_tensor(out=t4, in0=A, in1=sin_sb[:, sl], op=OP.mult)
                        nc.vector.tensor_tensor(out=ra, in0=t1, in1=t2, op=OP.subtract)
                        nc.vector.tensor_tensor(out=rb, in0=t3, in1=t4, op=OP.add)
                        # assemble QT/KT (dim order per head: [first32, second32]);
                        # KT gates all of attention -> fast DVE copies; QT is only
                        # needed per-qc (slack) -> slower GPSIMD copies are fine
                        nc.gpsimd.tensor_copy(out=QT[0:32, sl], in_=ra[0:32, :])
                        nc.gpsimd.tensor_copy(out=QT[32:64, sl], in_=rb[0:32, :])
                        nc.gpsimd.tensor_copy(out=QT[64:96, sl], in_=ra[32:64, :])
                        nc.gpsimd.tensor_copy(out=QT[96:128, sl], in_=rb[32:64, :])
                        nc.vector.tensor_copy(out=KT[0:32, sl], in_=ra[64:96, :])
                        nc.vector.tensor_copy(out=KT[32:64, sl], in_=rb[64:96, :])
                        nc.vector.tensor_copy(out=KT[64:96, sl], in_=ra[96:128, :])
                        nc.vector.tensor_copy(out=KT[96:128, sl], in_=rb[96:128, :])
                        # V natural GEMM for this n-range (+ mask fold)
                        for nb in range(nc8 * 4, nc8 * 4 + 4):
                            psv = opool.tile([128, 128], F32, tag="o", name="psv")
                            for kc in range(4):
                                nc.tensor.matmul(psv, xnT[kc][:, nb * 128:(nb + 1) * 128],
                                                 w_sb[:, kc, 256:384],
                                                 start=(kc == 0), stop=False)
                            nc.tensor.matmul(psv, ones_row[0:1, nb * 128:(nb + 1) * 128],
                                             w_sb[0:1, 4, 256:384], start=False, stop=True)
                            vdst = Vnat[:, nb, :].rearrange("p (g c) -> p g c", c=65)[:, :, 0:64]
                            vsrc = psv.rearrange("p (g c) -> p g c", c=64)
                            nc.vector.tensor_scalar(out=vdst, in0=vsrc,
                                                    scalar1=msk_sb[:, nb:nb + 1], scalar2=None,
                                                    op0=OP.mult, op1=OP.bypass)

            # ---------------- P6: attention
            # high_priority: let the scheduler interleave attention S/exp into
            # the engine order as soon as data deps allow, instead of queueing
            # them behind the whole preamble (head-of-line on PE/ACT)
            opks = []
            with tc.high_priority():
                for qc in range(QC):
                    qsl = slice(qc * 512, (qc + 1) * 512)
                    opk = okpool.tile([128, 512], BF16, tag="ok", name=f"opk{qc}")
                    for h in range(2):
                        hsl = slice(h * 64, (h + 1) * 64)
                        qh = QT[hsl, qsl]
                        o_ps = opool.tile([65, 512], F32, tag="o")
                        for g in range(NGRP):
                            sgrp = spool.tile([128, 1536], F32, tag="s")
                            for j in range(3):
                                mc = 3 * g + j
                                js = slice(j * 512, (j + 1) * 512)
                                if mc < 32:
                                    nc.tensor.matmul(sgrp[:, js],
                                                     KT[hsl, mc * 128:(mc + 1) * 128],
                                                     qh, start=True, stop=True)
                                else:
                                    nc.tensor.matmul(sgrp[0:NPM, js],
                                                     KT[hsl, N:M_TOT],
                                                     qh, start=True, stop=True)
                            pgrp = ppool.tile([128, 1536], BF16, tag="p")
                            # last group's pm slice has 112 never-written psum rows;
                            # exp of stale-but-finite logits there is never read
                            # (AV contracts only rows 0:16 of the pm slice).
                            nc.scalar.activation(out=pgrp, in_=sgrp, func=AF.Exp, scale=SCALE)
                            for j in range(3):
                                mc = 3 * g + j
                                js = slice(j * 512, (j + 1) * 512)
                                if mc < 32:
                                    nc.tensor.matmul(o_ps, Vnat[:, mc, 65 * h:65 * h + 65],
                                                     pgrp[:, js],
                                                     start=(mc == 0), stop=(mc == 32))
                                else:
                                    nc.tensor.matmul(o_ps, Vnat[0:NPM, mc, 65 * h:65 * h + 65],
                                                     pgrp[0:NPM, js],
                                                     start=False, stop=True)
                        # normalize: r = 1/denom ; broadcast via DRAM bounce ; opk_h = numer * r
                        r_sb = rpool.tile([1, 512], F32, tag="r")
                        nc.vector.reciprocal(out=r_sb, in_=o_ps[64:65, :])
                        r_dr = rdram.tile([1, 512], F32, tag="rd")
                        nc.sync.dma_start(out=r_dr[:, :], in_=r_sb)
                        rd_ap = r_dr[:, :]
                        r_bc = bass.AP(tensor=rd_ap.tensor, offset=rd_ap.offset,
                                       ap=[[0, 64]] + list(rd_ap.ap[1:]))
                        r64 = rbpool.tile([64, 512], F32, tag="rb")
                        nc.gpsimd.dma_start(out=r64, in_=r_bc)
                        nc.vector.tensor_tensor(out=opk[h * 64:(h + 1) * 64, :],
                                                in0=o_ps[0:64, :], in1=r64, op=OP.mult)
                    opks.append(opk)

            # ---------------- P7: output projection (K split per head)
            if True:
                for qc in range(QC):
                    for mb in range(4):
                        tag = "s" if (qc * 4 + mb) % 2 == 0 else "o"
                        psf = spool.tile([128, 512], F32, tag=tag, name="psf") \
                            if tag == "s" else opool.tile([128, 512], F32, tag=tag, name="psf")
                        nc.tensor.matmul(psf, opks[qc][:, mb * 128:(mb + 1) * 128],
                                         wo_sb, start=True, stop=True)
                        f_sb = fpool.tile([128, 512], F32, tag="fs")
                        if (qc * 4 + mb) % 2 == 0:
                            nc.scalar.copy(out=f_sb, in_=psf)
                        else:
                            nc.vector.tensor_copy(out=f_sb, in_=psf)
                        row0 = qc * 512 + mb * 128
                        eng = nc.sync if mb % 2 == 0 else nc.gpsimd
                        eng.dma_start(out=out_p[row0:row0 + 128, :], in_=f_sb)

    _split_excess_waits(nc)
    return nc


_STATE = {}


def _get_nc():
    if "nc" not in _STATE:
        _STATE["nc"] = build()
    return _STATE["nc"]


def _rope_tables():
    inv = 1.0 / (BASE ** (np.arange(0, DH, 2, dtype=np.float64) / DH))  # [32]
    pos = np.arange(N, dtype=np.float64)
    fr = pos[None, :] * inv[:, None]                                   # [32, N]
    cos32 = np.cos(fr)
    sin32 = np.sin(fr)
    cosf = np.tile(cos32, (4, 1)).astype(ml_dtypes.bfloat16)
    sinf = np.tile(sin32, (4, 1)).astype(ml_dtypes.bfloat16)
    return cosf, sinf


def kernel(x, mask, ln_w, ln_b, w_qkv, w_out, pm):
    bf = ml_dtypes.bfloat16
    f = np.float32
    x = np.asarray(x, f)
    mask_b = np.asarray(mask).astype(bool)
    ln_w = np.asarray(ln_w, f)
    ln_b = np.asarray(ln_b, f)
    w_qkv = np.asarray(w_qkv, f)
    w_out = np.asarray(w_out, f)
    pm = np.asarray(pm, f)

    cosf, sinf = _rope_tables()
    w_eff = ln_w[:, None] * w_qkv                     # [512, 1536]
    brow = ln_b @ w_qkv                               # [1536]

    in_maps = []
    for c in range(NCORES):
        bc = c // 4
        g0, g1 = (c % 4) * 2, (c % 4) * 2 + 1
        qcols = lambda g, lo, hi: np.arange(g * 64 + lo, g * 64 + hi)
        acols = np.concatenate([qcols(g0, 0, 32), qcols(g1, 0, 32),
                                512 + qcols(g0, 0, 32), 512 + qcols(g1, 0, 32)])
        bcols = np.concatenate([qcols(g0, 32, 64), qcols(g1, 32, 64),
                                512 + qcols(g0, 32, 64), 512 + qcols(g1, 32, 64)])
        vcols = np.concatenate([1024 + qcols(g0, 0, 64), 1024 + qcols(g1, 0, 64)])
        cols = np.concatenate([acols, bcols, vcols])  # [384]
        wq = np.zeros((5 * 128, 384), f)
        wq[0:512] = w_eff[:, cols]
        wq[512] = brow[cols]
        # pm[0, g] : [NPM, 64] -> K^T rows = dims, cols = pm idx
        pmk = np.concatenate([pm[0, g0].T, pm[0, g1].T], axis=0)  # [128, NPM]
        pmv = np.zeros((NPM, 130), f)
        pmv[:, 0:64] = pm[1, g0]
        pmv[:, 64] = 1.0
        pmv[:, 65:129] = pm[1, g1]
        pmv[:, 129] = 1.0
        mk = mask_b[bc, 0].astype(f).reshape(NBLK, 128).T  # [128, NBLK]
        in_maps.append(dict(
            x_in=x[bc].astype(bf),
            wq_in=wq.astype(bf),
            wo0_in=w_out[g0 * 64:(g0 + 1) * 64].astype(bf),
            wo1_in=w_out[g1 * 64:(g1 + 1) * 64].astype(bf),
            cos_in=cosf, sin_in=sinf,
            pmk_in=pmk.astype(bf), pmv_in=pmv.astype(bf),
            msk_in=np.ascontiguousarray(mk),
            ones_in=np.ones((1, N), bf),
        ))

    global _LAST_IN_MAPS
    _LAST_IN_MAPS = in_maps
    nc = _get_nc()
    res = run_bass_kernel_spmd(nc, in_maps, core_ids=list(range(NCORES)))
    out = np.zeros((B, N, D), f)
    for c in range(NCORES):
        out[c // 4] += res.results[c]["out_p"]
    return out
